# revision 6
# baseline (speedup 1.0000x reference)
"""ApproxSiLU16_FXP Trainium2 kernel (8 NeuronCores, data-parallel).

The reference computes a 16-segment piecewise-linear fixed-point
approximation of SiLU on a uniform knot grid t_k = -8 + 0.875k
(k = 0..16), with knot values y_k = round(1024*silu(t_k))/1024.
Instead of gathering from the LUT per element, this kernel
reconstructs the same piecewise-linear function analytically:

    u  = x*(8/7) + 64/7            (segment coordinate, in [0,16])
    k  = floor(u)                  (magic-constant floor: +2^23-0.5)
    fr = u - k
    out = silu(t_k) + fr*(silu(t_k+0.875) - silu(t_k))

using the ScalarEngine's Silu activation for the knot values.  This
matches the fixed-point reference to ~1e-3 relative error (the only
differences are the reference's int rounding of the LUT entries / the
interpolation weight, and bf16 rounding of the blend, all well under
the 2e-2 gate).

Engine split per tile (to balance against the ~50 MB/core DMA):
  DVE : u, kfm (fp32 tensor_scalar, 2x), nfr (fused STT), g, o (bf16 2x)
  ACT : kz = 0.875*kfm - (0.875*2^23+8) via Copy-FMA; a = silu(kz);
        b = silu(kz + 0.875)  (both written bf16)
  POOL: t = a - b  (bf16 tensor_tensor)
  out = a + (a-b)*(k-u) = a + fr*(b-a), stored bf16.

Sharding: x is (8, 2048, 4096); core i processes batch row i.
"""

import numpy as np

from concourse import bacc, mybir
import concourse.tile as tile
from concourse.bass_utils import run_bass_kernel_spmd

F32 = mybir.dt.float32
BF16 = mybir.dt.bfloat16
Alu = mybir.AluOpType
Act = mybir.ActivationFunctionType

P = 128          # SBUF partitions
FD = 2048        # free dim per tile
NT = 32          # tiles per core shard: 2048*4096 = NT*P*FD
N_CORES = 8

MA = 8388607.5   # 2^23 - 0.5  (magic floor, round-to-nearest-even)
MB = -8388608.0  # -2^23
C87 = float(8.0 / 7.0)
C647 = float(64.0 / 7.0)
KZ_BIAS = float(-(0.875 * 8388608.0 + 8.0))   # -7340040, exactly representable


def _reg_const(nc, val):
    t = nc.alloc_sbuf_tensor(f"const-f32-{val}", [128, 1], F32)
    nc.gpsimd.memset(t.ap(), val)
    nc.const_aps.aps[(F32, float(val))] = t.ap()


def build():
    nc = bacc.Bacc()
    _reg_const(nc, 0.875)
    nc.all_engine_barrier()
    x_ext = nc.declare_dram_parameter("x", [NT, P, FD], F32, isOutput=False)
    o_ext = nc.declare_dram_parameter("out", [NT, P, FD], BF16, isOutput=True)

    with tile.TileContext(nc) as tc, tc.tile_pool(name="p", bufs=3) as pool:
        for i in range(NT):
            xt = pool.tile([P, FD], F32, tag="xt")
            nc.sync.dma_start(xt[:], x_ext[i])
            # u = x*(8/7) + 64/7
            u = pool.tile([P, FD], F32, tag="u")
            nc.vector.tensor_scalar(u[:], xt[:], C87, C647, Alu.mult, Alu.add)
            # kfm = 2^23 + min(floor(u), 16)
            kfm = pool.tile([P, FD], F32, tag="kfm")
            nc.vector.tensor_scalar(kfm[:], u[:], 16.0, MA, Alu.min, Alu.add)
            # kz = 0.875*kfm - (0.875*2^23 + 8) = t_k  (ACT Copy FMA, exact)
            kz = pool.tile([P, FD], F32, tag="kz")
            nc.scalar.activation(kz[:], kfm[:], Act.Copy, bias=KZ_BIAS, scale=0.875)
            # nfr875 = kz - x = 0.875*(k - u) = -0.875*fr   (bf16 out)
            nfr = pool.tile([P, FD], BF16, tag="nfr")
            nc.vector.tensor_tensor(nfr[:], kz[:], xt[:], Alu.subtract)
            # a = silu(t_k), b = silu(t_k + 0.875)   (bf16 out)
            a = pool.tile([P, FD], BF16, tag="a")
            nc.scalar.activation(a[:], kz[:], Act.Silu)
            b = pool.tile([P, FD], BF16, tag="b")
            nc.scalar.activation(b[:], kz[:], Act.Silu, bias=0.875)
            # t = a - b   (POOL, bf16)
            t = pool.tile([P, FD], BF16, tag="t")
            nc.gpsimd.tensor_tensor(t[:], a[:], b[:], Alu.subtract)
            # g875 = t*nfr875 = 0.875*(b-a)*fr   (bf16 2x)
            g875 = pool.tile([P, FD], BF16, tag="g875")
            nc.vector.tensor_tensor(g875[:], t[:], nfr[:], Alu.mult)
            # g = g875*(8/7) = (b-a)*fr   (bf16 single-src 4x)
            g = pool.tile([P, FD], BF16, tag="g")
            nc.vector.tensor_scalar(g[:], g875[:], C87, None, Alu.mult)
            # o = a + g   (bf16 2x)
            o = pool.tile([P, FD], BF16, tag="o")
            nc.vector.tensor_tensor(o[:], a[:], g[:], Alu.add)
            nc.sync.dma_start(o_ext[i], o[:])
    nc.compile()
    return nc


_NC_CACHE = None


def _get_nc():
    global _NC_CACHE
    if _NC_CACHE is None:
        _NC_CACHE = build()
    return _NC_CACHE


def _ensure_ntff_hook():
    """Install the antenv.axon_hooks shim so trace=True works under axon."""
    import sys
    import types

    if "antenv.axon_hooks" not in sys.modules:
        mod = types.ModuleType("antenv.axon_hooks")
        _h = [None]
        mod.set_axon_ntff_profile_hook = lambda h: _h.__setitem__(0, h)
        mod.get_axon_ntff_profile_hook = lambda: _h[0]
        sys.modules["antenv.axon_hooks"] = mod
        import antenv

        antenv.axon_hooks = mod
    import antenv.axon_hooks as ah

    if ah.get_axon_ntff_profile_hook() is None:
        from trn_agent_boot.trn_boot import _ntff_profile_via_ctypes

        h = _ntff_profile_via_ctypes("/opt/axon/libaxon_pjrt.so")
        if h is not None:
            ah.set_axon_ntff_profile_hook(h)
    # avoid cloud artifact uploads in this container
    import concourse.bass_utils as bu

    bu.upload_artifacts = lambda tmpdir: tmpdir


def _run(x, trace=False, trace_kwargs=None):
    """x: (8, 2048, 4096) float32. Returns (out, exec_time_ns|None)."""
    x = np.ascontiguousarray(np.asarray(x, dtype=np.float32))
    assert x.shape == (N_CORES, 2048, 4096), x.shape
    nc = _get_nc()
    core_ids = list(range(N_CORES))
    in_maps = [{"x": x[i].reshape(NT, P, FD)} for i in range(N_CORES)]
    kwargs = {}
    if trace:
        _ensure_ntff_hook()
        kwargs["trace"] = True
        if trace_kwargs:
            kwargs.update(trace_kwargs)
    res = run_bass_kernel_spmd(nc, in_maps, core_ids, **kwargs)
    out = np.empty((N_CORES, 2048, 4096), dtype=np.float32)
    for i in range(N_CORES):
        out[i] = np.asarray(res.results[i]["out"], dtype=np.float32).reshape(
            2048, 4096
        )
    return out, res.exec_time_ns


def kernel(x, seg=None, silu_vals=None, **_unused):
    out, _ = _run(x, trace=False)
    return out


# revision 7
# speedup vs baseline: 1.4630x; 1.4630x over previous
"""ApproxSiLU16_FXP Trainium2 kernel (8 NeuronCores, data-parallel).

The reference computes a 16-segment piecewise-linear fixed-point
approximation of SiLU on a uniform knot grid t_k = -8 + 0.875k
(k = 0..16), with knot values y_k = round(1024*silu(t_k))/1024.
Instead of gathering from the LUT per element, this kernel
reconstructs the same piecewise-linear function analytically:

    u  = x*(8/7) + 64/7            (segment coordinate, in [0,16])
    k  = floor(u)                  (magic-constant floor: +2^23-0.5)
    fr = u - k
    out = silu(t_k) + fr*(silu(t_k+0.875) - silu(t_k))

using the ScalarEngine's Silu activation for the knot values.  This
matches the fixed-point reference to ~1e-3 relative error (the only
differences are the reference's int rounding of the LUT entries / the
interpolation weight, and bf16 rounding of the blend, all well under
the 2e-2 gate).

Engine split per tile (to balance against the ~50 MB/core DMA):
  DVE : u, kfm (fp32 tensor_scalar, 2x), nfr (fused STT), g, o (bf16 2x)
  ACT : kz = 0.875*kfm - (0.875*2^23+8) via Copy-FMA; a = silu(kz);
        b = silu(kz + 0.875)  (both written bf16)
  POOL: t = a - b  (bf16 tensor_tensor)
  out = a + (a-b)*(k-u) = a + fr*(b-a), stored bf16.

Sharding: x is (8, 2048, 4096); core i processes batch row i.
"""

import numpy as np

from concourse import bacc, mybir
import concourse.tile as tile
from concourse.bass_utils import run_bass_kernel_spmd

F32 = mybir.dt.float32
BF16 = mybir.dt.bfloat16
Alu = mybir.AluOpType
Act = mybir.ActivationFunctionType

P = 128          # SBUF partitions
FD = 2048        # free dim per tile
NT = 32          # tiles per core shard: 2048*4096 = NT*P*FD
N_CORES = 8

MA = 8388607.5   # 2^23 - 0.5  (magic floor, round-to-nearest-even)
MB = -8388608.0  # -2^23
C87 = float(8.0 / 7.0)
C647 = float(64.0 / 7.0)
KZ_BIAS = float(-(0.875 * 8388608.0 + 8.0))   # -7340040, exactly representable


def _reg_const(nc, val):
    t = nc.alloc_sbuf_tensor(f"const-f32-{val}", [128, 1], F32)
    nc.gpsimd.memset(t.ap(), val)
    nc.const_aps.aps[(F32, float(val))] = t.ap()


def build():
    nc = bacc.Bacc()
    _reg_const(nc, 0.875)
    nc.all_engine_barrier()
    x_ext = nc.declare_dram_parameter("x", [NT, P, FD], F32, isOutput=False)
    o_ext = nc.declare_dram_parameter("out", [NT, P, FD], BF16, isOutput=True)

    with tile.TileContext(nc) as tc, tc.tile_pool(name="p", bufs=3) as pool:
        for i in range(NT):
            xt = pool.tile([P, FD], F32, tag="xt")
            nc.sync.dma_start(xt[:], x_ext[i])
            # u = x*(8/7) + 64/7
            u = pool.tile([P, FD], F32, tag="u")
            nc.vector.tensor_scalar(u[:], xt[:], C87, C647, Alu.mult, Alu.add)
            # kfm = 2^23 + min(floor(u), 16)
            kfm = pool.tile([P, FD], F32, tag="kfm")
            nc.vector.tensor_scalar(kfm[:], u[:], 16.0, MA, Alu.min, Alu.add)
            # ktile = k   (fp32 single-src TS, 2x)
            ktile = pool.tile([P, FD], F32, tag="ktile")
            nc.vector.tensor_scalar(ktile[:], kfm[:], MB, None, Alu.add)
            # kz = 0.875*kfm - (0.875*2^23 + 8) = t_k  (ACT Copy FMA, exact)
            kz = pool.tile([P, FD], F32, tag="kz")
            nc.scalar.activation(kz[:], kfm[:], Act.Copy, bias=KZ_BIAS, scale=0.875)
            # fr = u - k   (bf16 out)
            fr = pool.tile([P, FD], BF16, tag="fr")
            nc.vector.tensor_tensor(fr[:], u[:], ktile[:], Alu.subtract)
            # a = silu(t_k), b = silu(t_k + 0.875)   (bf16 out)
            a = pool.tile([P, FD], BF16, tag="a")
            nc.scalar.activation(a[:], kz[:], Act.Silu)
            b = pool.tile([P, FD], BF16, tag="b")
            nc.scalar.activation(b[:], kz[:], Act.Silu, bias=0.875)
            # t = a - b   (bf16 2x)
            t = pool.tile([P, FD], BF16, tag="t")
            nc.vector.tensor_tensor(t[:], a[:], b[:], Alu.subtract)
            # g = t*fr = (a-b)*fr   (bf16 2x)
            g = pool.tile([P, FD], BF16, tag="g")
            nc.vector.tensor_tensor(g[:], t[:], fr[:], Alu.mult)
            # o = a - g = a + (b-a)*fr   (bf16 2x)
            o = pool.tile([P, FD], BF16, tag="o")
            nc.vector.tensor_tensor(o[:], a[:], g[:], Alu.subtract)
            nc.sync.dma_start(o_ext[i], o[:])
    nc.compile()
    return nc


_NC_CACHE = None


def _get_nc():
    global _NC_CACHE
    if _NC_CACHE is None:
        _NC_CACHE = build()
    return _NC_CACHE


def _ensure_ntff_hook():
    """Install the antenv.axon_hooks shim so trace=True works under axon."""
    import sys
    import types

    if "antenv.axon_hooks" not in sys.modules:
        mod = types.ModuleType("antenv.axon_hooks")
        _h = [None]
        mod.set_axon_ntff_profile_hook = lambda h: _h.__setitem__(0, h)
        mod.get_axon_ntff_profile_hook = lambda: _h[0]
        sys.modules["antenv.axon_hooks"] = mod
        import antenv

        antenv.axon_hooks = mod
    import antenv.axon_hooks as ah

    if ah.get_axon_ntff_profile_hook() is None:
        from trn_agent_boot.trn_boot import _ntff_profile_via_ctypes

        h = _ntff_profile_via_ctypes("/opt/axon/libaxon_pjrt.so")
        if h is not None:
            ah.set_axon_ntff_profile_hook(h)
    # avoid cloud artifact uploads in this container
    import concourse.bass_utils as bu

    bu.upload_artifacts = lambda tmpdir: tmpdir


def _run(x, trace=False, trace_kwargs=None):
    """x: (8, 2048, 4096) float32. Returns (out, exec_time_ns|None)."""
    x = np.ascontiguousarray(np.asarray(x, dtype=np.float32))
    assert x.shape == (N_CORES, 2048, 4096), x.shape
    nc = _get_nc()
    core_ids = list(range(N_CORES))
    in_maps = [{"x": x[i].reshape(NT, P, FD)} for i in range(N_CORES)]
    kwargs = {}
    if trace:
        _ensure_ntff_hook()
        kwargs["trace"] = True
        if trace_kwargs:
            kwargs.update(trace_kwargs)
    res = run_bass_kernel_spmd(nc, in_maps, core_ids, **kwargs)
    out = np.empty((N_CORES, 2048, 4096), dtype=np.float32)
    for i in range(N_CORES):
        out[i] = np.asarray(res.results[i]["out"], dtype=np.float32).reshape(
            2048, 4096
        )
    return out, res.exec_time_ns


def kernel(x, seg=None, silu_vals=None, **_unused):
    out, _ = _run(x, trace=False)
    return out


# revision 9
# speedup vs baseline: 1.6762x; 1.1457x over previous
"""ApproxSiLU16_FXP Trainium2 kernel (8 NeuronCores, data-parallel).

The reference computes a 16-segment piecewise-linear fixed-point
approximation of SiLU on a uniform knot grid t_k = -8 + 0.875k
(k = 0..16), with knot values y_k = round(1024*silu(t_k))/1024.
Instead of gathering from the LUT per element, this kernel
reconstructs the same piecewise-linear function analytically:

    u  = x*(8/7) + 64/7            (segment coordinate, in [0,16])
    k  = floor(u)                  (magic-constant floor: +2^23-0.5)
    fr = u - k
    out = silu(t_k) + fr*(silu(t_k+0.875) - silu(t_k))

using the ScalarEngine's Silu activation for the knot values.  This
matches the fixed-point reference to ~1e-3 relative error (the only
differences are the reference's int rounding of the LUT entries / the
interpolation weight, and bf16 rounding of the blend, all well under
the 2e-2 gate).

Engine split per tile (to balance against the ~50 MB/core DMA):
  DVE : u, kfm (fp32 tensor_scalar, 2x), nfr (fused STT), g, o (bf16 2x)
  ACT : kz = 0.875*kfm - (0.875*2^23+8) via Copy-FMA; a = silu(kz);
        b = silu(kz + 0.875)  (both written bf16)
  POOL: t = a - b  (bf16 tensor_tensor)
  out = a + (a-b)*(k-u) = a + fr*(b-a), stored bf16.

Sharding: x is (8, 2048, 4096); core i processes batch row i.
"""

import numpy as np

from concourse import bacc, mybir
import concourse.tile as tile
from concourse.bass_utils import run_bass_kernel_spmd

F32 = mybir.dt.float32
BF16 = mybir.dt.bfloat16
Alu = mybir.AluOpType
Act = mybir.ActivationFunctionType

P = 128          # SBUF partitions
FD = 2048        # free dim per tile
NT = 32          # tiles per core shard: 2048*4096 = NT*P*FD
N_CORES = 8

MA = 8388607.5   # 2^23 - 0.5  (magic floor, round-to-nearest-even)
MB = -8388608.0  # -2^23
C87 = float(8.0 / 7.0)
C647 = float(64.0 / 7.0)
KZ_BIAS = float(-(0.875 * 8388608.0 + 8.0))   # -7340040, exactly representable


def _reg_const(nc, val):
    t = nc.alloc_sbuf_tensor(f"const-f32-{val}", [128, 1], F32)
    nc.gpsimd.memset(t.ap(), val)
    nc.const_aps.aps[(F32, float(val))] = t.ap()


_FR_OP = None


def _get_fr_op():
    """Custom DVE op: fr = (in0*C1 + C2) - (in1 + C0)  [= u - k], one pass."""
    global _FR_OP
    if _FR_OP is not None:
        return _FR_OP
    import concourse.dve_ops as dve_ops
    from concourse.dve_spec import Spec, Src0, Src1, C0, C1, C2, lower, _has_src1
    from concourse.dve_uop import DveOpSpec

    name = "FR_FROM_MAGIC_ANT"
    body = (Src0 * C1 + C2) - (Src1 + C0)
    spec = Spec(
        body=body,
        reference=lambda in0, in1, s0, s1, imm2: (in0 * s1 + imm2) - (in1 + s0),
    )
    op = dve_ops.DveOp(name, spec, subdim=False, uops_sha={})
    for ver in ("v3", "v4"):
        s = DveOpSpec(
            name=name,
            opcode=1,  # placeholder; sha only covers uop bytes
            uops=lower(spec, ver=ver),
            rd1_en=_has_src1(spec),
        )
        op.uops_sha[ver] = s.sha(ver)
    if name not in dve_ops._SUB_OPCODE_FOR_NAME:
        dve_ops.OPS.append(op)
        dve_ops._SUB_OPCODE_FOR_NAME[name] = (
            dve_ops._CUSTOM_DVE_ROW_BASE + len(dve_ops.OPS) - 1
        )
        dve_ops.CUSTOM_DVE_SPECS[name] = spec
    assert dve_ops._SUB_OPCODE_FOR_NAME[name] < 0x20
    _FR_OP = op
    return op


def build():
    nc = bacc.Bacc()
    _reg_const(nc, 0.875)
    nc.all_engine_barrier()
    x_ext = nc.declare_dram_parameter("x", [NT, P, FD], F32, isOutput=False)
    o_ext = nc.declare_dram_parameter("out", [NT, P, FD], BF16, isOutput=True)

    with tile.TileContext(nc) as tc, tc.tile_pool(name="p", bufs=3) as pool:
        for i in range(NT):
            xt = pool.tile([P, FD], F32, tag="xt")
            nc.sync.dma_start(xt[:], x_ext[i])
            # u = x*(8/7) + 64/7
            u = pool.tile([P, FD], F32, tag="u")
            nc.vector.tensor_scalar(u[:], xt[:], C87, C647, Alu.mult, Alu.add)
            # kfm = 2^23 + min(floor(u), 16)
            kfm = pool.tile([P, FD], F32, tag="kfm")
            nc.vector.tensor_scalar(kfm[:], u[:], 16.0, MA, Alu.min, Alu.add)
            # kz = 0.875*kfm - (0.875*2^23 + 8) = t_k  (ACT Copy FMA, exact)
            kz = pool.tile([P, FD], F32, tag="kz")
            nc.scalar.activation(kz[:], kfm[:], Act.Copy, bias=KZ_BIAS, scale=0.875)
            # fr = (x*(8/7) + 64/7) - (kfm - 2^23) = u - k   (custom DVE, bf16 out)
            fr = pool.tile([P, FD], BF16, tag="fr")
            nc.vector._custom_dve(
                _get_fr_op(), out=fr[:], in0=xt[:], in1=kfm[:], s0=MB, s1=C87, imm2=C647
            )
            # a = silu(t_k), b = silu(t_k + 0.875)   (bf16 out)
            a = pool.tile([P, FD], BF16, tag="a")
            nc.scalar.activation(a[:], kz[:], Act.Silu)
            b = pool.tile([P, FD], BF16, tag="b")
            nc.scalar.activation(b[:], kz[:], Act.Silu, bias=0.875)
            # t = a - b   (bf16 2x)
            t = pool.tile([P, FD], BF16, tag="t")
            nc.vector.tensor_tensor(t[:], a[:], b[:], Alu.subtract)
            # g = t*fr = (a-b)*fr   (bf16 2x)
            g = pool.tile([P, FD], BF16, tag="g")
            nc.vector.tensor_tensor(g[:], t[:], fr[:], Alu.mult)
            # o = a - g = a + (b-a)*fr   (bf16 2x)
            o = pool.tile([P, FD], BF16, tag="o")
            nc.vector.tensor_tensor(o[:], a[:], g[:], Alu.subtract)
            nc.sync.dma_start(o_ext[i], o[:])
    nc.compile()
    return nc


_NC_CACHE = None


def _get_nc():
    global _NC_CACHE
    if _NC_CACHE is None:
        _NC_CACHE = build()
    return _NC_CACHE


def _ensure_ntff_hook():
    """Install the antenv.axon_hooks shim so trace=True works under axon."""
    import sys
    import types

    if "antenv.axon_hooks" not in sys.modules:
        mod = types.ModuleType("antenv.axon_hooks")
        _h = [None]
        mod.set_axon_ntff_profile_hook = lambda h: _h.__setitem__(0, h)
        mod.get_axon_ntff_profile_hook = lambda: _h[0]
        sys.modules["antenv.axon_hooks"] = mod
        import antenv

        antenv.axon_hooks = mod
    import antenv.axon_hooks as ah

    if ah.get_axon_ntff_profile_hook() is None:
        from trn_agent_boot.trn_boot import _ntff_profile_via_ctypes

        h = _ntff_profile_via_ctypes("/opt/axon/libaxon_pjrt.so")
        if h is not None:
            ah.set_axon_ntff_profile_hook(h)
    # avoid cloud artifact uploads in this container
    import concourse.bass_utils as bu

    bu.upload_artifacts = lambda tmpdir: tmpdir


def _run(x, trace=False, trace_kwargs=None):
    """x: (8, 2048, 4096) float32. Returns (out, exec_time_ns|None)."""
    x = np.ascontiguousarray(np.asarray(x, dtype=np.float32))
    assert x.shape == (N_CORES, 2048, 4096), x.shape
    nc = _get_nc()
    core_ids = list(range(N_CORES))
    in_maps = [{"x": x[i].reshape(NT, P, FD)} for i in range(N_CORES)]
    kwargs = {}
    if trace:
        _ensure_ntff_hook()
        kwargs["trace"] = True
        if trace_kwargs:
            kwargs.update(trace_kwargs)
    res = run_bass_kernel_spmd(nc, in_maps, core_ids, **kwargs)
    out = np.empty((N_CORES, 2048, 4096), dtype=np.float32)
    for i in range(N_CORES):
        out[i] = np.asarray(res.results[i]["out"], dtype=np.float32).reshape(
            2048, 4096
        )
    return out, res.exec_time_ns


def kernel(x, seg=None, silu_vals=None, **_unused):
    out, _ = _run(x, trace=False)
    return out


# revision 10
# speedup vs baseline: 1.7732x; 1.0579x over previous
"""ApproxSiLU16_FXP Trainium2 kernel (8 NeuronCores, data-parallel).

The reference computes a 16-segment piecewise-linear fixed-point
approximation of SiLU on a uniform knot grid t_k = -8 + 0.875k
(k = 0..16), with knot values y_k = round(1024*silu(t_k))/1024.
Instead of gathering from the LUT per element, this kernel
reconstructs the same piecewise-linear function analytically:

    u  = x*(8/7) + 64/7            (segment coordinate, in [0,16])
    k  = floor(u)                  (magic-constant floor: +2^23-0.5)
    fr = u - k
    out = silu(t_k) + fr*(silu(t_k+0.875) - silu(t_k))

using the ScalarEngine's Silu activation for the knot values.  This
matches the fixed-point reference to ~1e-3 relative error (the only
differences are the reference's int rounding of the LUT entries / the
interpolation weight, and bf16 rounding of the blend, all well under
the 2e-2 gate).

Engine split per tile (to balance against the ~50 MB/core DMA):
  DVE : u, kfm (fp32 tensor_scalar, 2x), nfr (fused STT), g, o (bf16 2x)
  ACT : kz = 0.875*kfm - (0.875*2^23+8) via Copy-FMA; a = silu(kz);
        b = silu(kz + 0.875)  (both written bf16)
  POOL: t = a - b  (bf16 tensor_tensor)
  out = a + (a-b)*(k-u) = a + fr*(b-a), stored bf16.

Sharding: x is (8, 2048, 4096); core i processes batch row i.
"""

import numpy as np

from concourse import bacc, mybir
import concourse.tile as tile
from concourse.bass_utils import run_bass_kernel_spmd

F32 = mybir.dt.float32
BF16 = mybir.dt.bfloat16
Alu = mybir.AluOpType
Act = mybir.ActivationFunctionType

P = 128          # SBUF partitions
FD = 2048        # free dim per tile
NT = 32          # tiles per core shard: 2048*4096 = NT*P*FD
N_CORES = 8

MA = 8388607.5   # 2^23 - 0.5  (magic floor, round-to-nearest-even)
MB = -8388608.0  # -2^23
C87 = float(8.0 / 7.0)
C647 = float(64.0 / 7.0)
KZ_BIAS = float(-(0.875 * 8388608.0 + 8.0))   # -7340040, exactly representable


def _reg_const(nc, val):
    t = nc.alloc_sbuf_tensor(f"const-f32-{val}", [128, 1], F32)
    nc.gpsimd.memset(t.ap(), val)
    nc.const_aps.aps[(F32, float(val))] = t.ap()


_FR_OP = None


def _get_fr_op():
    """Custom DVE op: fr = (in0*C1 + C2) - (in1 + C0)  [= u - k], one pass."""
    global _FR_OP
    if _FR_OP is not None:
        return _FR_OP
    import concourse.dve_ops as dve_ops
    from concourse.dve_spec import Spec, Src0, Src1, C0, C1, C2, lower, _has_src1
    from concourse.dve_uop import DveOpSpec

    name = "FR_FROM_MAGIC_ANT"
    body = (Src0 * C1 + C2) - (Src1 + C0)
    spec = Spec(
        body=body,
        reference=lambda in0, in1, s0, s1, imm2: (in0 * s1 + imm2) - (in1 + s0),
    )
    op = dve_ops.DveOp(name, spec, subdim=False, uops_sha={})
    for ver in ("v3", "v4"):
        s = DveOpSpec(
            name=name,
            opcode=1,  # placeholder; sha only covers uop bytes
            uops=lower(spec, ver=ver),
            rd1_en=_has_src1(spec),
        )
        op.uops_sha[ver] = s.sha(ver)
    if name not in dve_ops._SUB_OPCODE_FOR_NAME:
        dve_ops.OPS.append(op)
        dve_ops._SUB_OPCODE_FOR_NAME[name] = (
            dve_ops._CUSTOM_DVE_ROW_BASE + len(dve_ops.OPS) - 1
        )
        dve_ops.CUSTOM_DVE_SPECS[name] = spec
    assert dve_ops._SUB_OPCODE_FOR_NAME[name] < 0x20
    _FR_OP = op
    return op


def build():
    nc = bacc.Bacc()
    _reg_const(nc, 0.875)
    nc.all_engine_barrier()
    x_ext = nc.declare_dram_parameter("x", [NT, P, FD], F32, isOutput=False)
    o_ext = nc.declare_dram_parameter("out", [NT, P, FD], BF16, isOutput=True)

    with tile.TileContext(nc) as tc, tc.tile_pool(name="p", bufs=3) as pool:
        for i in range(NT):
            xt = pool.tile([P, FD], F32, tag="xt")
            nc.sync.dma_start(xt[:], x_ext[i])
            # u = x*(8/7) + 64/7   (ACT Copy FMA — offloads the busier DVE)
            u = pool.tile([P, FD], F32, tag="u")
            nc.scalar.activation(u[:], xt[:], Act.Copy, bias=C647, scale=C87)
            # kfm = 2^23 + min(floor(u), 16)
            kfm = pool.tile([P, FD], F32, tag="kfm")
            nc.vector.tensor_scalar(kfm[:], u[:], 16.0, MA, Alu.min, Alu.add)
            # kz = 0.875*kfm - (0.875*2^23 + 8) = t_k   (exact both ways;
            # split between ACT Copy-FMA and DVE TS to balance engine load)
            kz = pool.tile([P, FD], F32, tag="kz")
            if i % 5 == 4:
                nc.vector.tensor_scalar(kz[:], kfm[:], MB, 0.875, Alu.add, Alu.mult)
            else:
                nc.scalar.activation(
                    kz[:], kfm[:], Act.Copy, bias=KZ_BIAS, scale=0.875
                )
            # fr = (x*(8/7) + 64/7) - (kfm - 2^23) = u - k   (custom DVE, bf16 out)
            fr = pool.tile([P, FD], BF16, tag="fr")
            nc.vector._custom_dve(
                _get_fr_op(), out=fr[:], in0=xt[:], in1=kfm[:], s0=MB, s1=C87, imm2=C647
            )
            # a = silu(t_k), b = silu(t_k + 0.875)   (bf16 out)
            a = pool.tile([P, FD], BF16, tag="a")
            nc.scalar.activation(a[:], kz[:], Act.Silu)
            b = pool.tile([P, FD], BF16, tag="b")
            nc.scalar.activation(b[:], kz[:], Act.Silu, bias=0.875)
            # t = a - b   (bf16 2x)
            t = pool.tile([P, FD], BF16, tag="t")
            nc.vector.tensor_tensor(t[:], a[:], b[:], Alu.subtract)
            # g = t*fr = (a-b)*fr   (bf16 2x)
            g = pool.tile([P, FD], BF16, tag="g")
            nc.vector.tensor_tensor(g[:], t[:], fr[:], Alu.mult)
            # o = a - g = a + (b-a)*fr   (bf16 2x)
            o = pool.tile([P, FD], BF16, tag="o")
            nc.vector.tensor_tensor(o[:], a[:], g[:], Alu.subtract)
            nc.sync.dma_start(o_ext[i], o[:])
    nc.compile()
    return nc


_NC_CACHE = None


def _get_nc():
    global _NC_CACHE
    if _NC_CACHE is None:
        _NC_CACHE = build()
    return _NC_CACHE


def _ensure_ntff_hook():
    """Install the antenv.axon_hooks shim so trace=True works under axon."""
    import sys
    import types

    if "antenv.axon_hooks" not in sys.modules:
        mod = types.ModuleType("antenv.axon_hooks")
        _h = [None]
        mod.set_axon_ntff_profile_hook = lambda h: _h.__setitem__(0, h)
        mod.get_axon_ntff_profile_hook = lambda: _h[0]
        sys.modules["antenv.axon_hooks"] = mod
        import antenv

        antenv.axon_hooks = mod
    import antenv.axon_hooks as ah

    if ah.get_axon_ntff_profile_hook() is None:
        from trn_agent_boot.trn_boot import _ntff_profile_via_ctypes

        h = _ntff_profile_via_ctypes("/opt/axon/libaxon_pjrt.so")
        if h is not None:
            ah.set_axon_ntff_profile_hook(h)
    # avoid cloud artifact uploads in this container
    import concourse.bass_utils as bu

    bu.upload_artifacts = lambda tmpdir: tmpdir


def _run(x, trace=False, trace_kwargs=None):
    """x: (8, 2048, 4096) float32. Returns (out, exec_time_ns|None)."""
    x = np.ascontiguousarray(np.asarray(x, dtype=np.float32))
    assert x.shape == (N_CORES, 2048, 4096), x.shape
    nc = _get_nc()
    core_ids = list(range(N_CORES))
    in_maps = [{"x": x[i].reshape(NT, P, FD)} for i in range(N_CORES)]
    kwargs = {}
    if trace:
        _ensure_ntff_hook()
        kwargs["trace"] = True
        if trace_kwargs:
            kwargs.update(trace_kwargs)
    res = run_bass_kernel_spmd(nc, in_maps, core_ids, **kwargs)
    out = np.empty((N_CORES, 2048, 4096), dtype=np.float32)
    for i in range(N_CORES):
        out[i] = np.asarray(res.results[i]["out"], dtype=np.float32).reshape(
            2048, 4096
        )
    return out, res.exec_time_ns


def kernel(x, seg=None, silu_vals=None, **_unused):
    out, _ = _run(x, trace=False)
    return out


# revision 18
# speedup vs baseline: 2.0108x; 1.1339x over previous
"""ApproxSiLU16_FXP Trainium2 kernel (8 NeuronCores, data-parallel).

The reference computes a 16-segment piecewise-linear fixed-point
approximation of SiLU on a uniform knot grid t_k = -8 + 0.875k
(k = 0..16), with knot values y_k = round(1024*silu(t_k))/1024.
Instead of gathering from the LUT per element, this kernel
reconstructs the same piecewise-linear function analytically:

    u   = x*(8/7) + 64/7          (segment coordinate, in [0,16])
    k   = floor(u)
    fr  = u - k
    out = silu(t_k) + fr*(silu(t_k+0.875) - silu(t_k))

using the ScalarEngine's Silu activation for the knot values.  This
matches the fixed-point reference to ~3e-3 relative error (reference
LUT int rounding + bf16 rounding of the blend), well under the 2e-2
gate.

floor() uses a small-magic bf16 trick: u' = u - 0.5 rounded to bf16,
then kfm = bf16(u' + 141).  141 + [-0.5, 16] lies inside the bf16
binade [128, 256) where the bf16 ulp is exactly 1.0, so the output
rounding snaps to 141 + floor(u).  Both Silu reads then use the free
input FMA: t_k = 0.875*kfm - 131.375 (exact in fp32), which keeps the
whole front-end in cheap bf16 ops and needs no fp32 magic tensor.
fr is recovered at full precision by a custom DVE op
fr = (x*(8/7) + 64/7) - (kfm - 141) reading the original fp32 x.

Engine split per tile:
  ACT : u' (Copy FMA -> bf16), a = silu(...), b = silu(...)
  DVE : kfm (bf16 1-scalar add), fr (custom), t, g, o (bf16 2x TT)
out = a - (a-b)*fr, stored bf16; the host upcasts to fp32.

Sharding: x is (8, 2048, 4096); core i processes batch row i.
"""

import numpy as np

from concourse import bacc, mybir
import concourse.tile as tile
from concourse.bass_utils import run_bass_kernel_spmd

F32 = mybir.dt.float32
BF16 = mybir.dt.bfloat16
Alu = mybir.AluOpType
Act = mybir.ActivationFunctionType

P = 128          # SBUF partitions
FD = 2048        # free dim per tile
NT = 32          # tiles per core shard: 2048*4096 = NT*P*FD
N_CORES = 8

C87 = float(8.0 / 7.0)
C647 = float(64.0 / 7.0)
UP_BIAS = float(64.0 / 7.0 - 0.5)
MAGIC = 141.0          # 141 - 0.5 + u in [140.5, 156.6] (bf16 ulp = 1)
BIAS_A = float(0.875 * MAGIC - 131.375 - 0.875 * MAGIC)  # placeholder, see below
SILU_BIAS_A = -131.375  # 0.875*(k+141) - 131.375 = 0.875k - 8 = t_k
SILU_BIAS_B = -130.5    # t_k + 0.875


_FR_OP = None


def _get_fr_op():
    """Custom DVE op: fr = (in0*C1 + C2) - (in1 + C0)  [= u - k], one pass."""
    global _FR_OP
    if _FR_OP is not None:
        return _FR_OP
    import concourse.dve_ops as dve_ops
    from concourse.dve_spec import Spec, Src0, Src1, C0, C1, C2, lower, _has_src1
    from concourse.dve_uop import DveOpSpec

    name = "FR_FROM_MAGIC_ANT"
    body = (Src0 * C1 + C2) - (Src1 + C0)
    spec = Spec(
        body=body,
        reference=lambda in0, in1, s0, s1, imm2: (in0 * s1 + imm2) - (in1 + s0),
    )
    op = dve_ops.DveOp(name, spec, subdim=False, uops_sha={})
    for ver in ("v3", "v4"):
        s = DveOpSpec(
            name=name,
            opcode=1,  # placeholder; sha only covers uop bytes
            uops=lower(spec, ver=ver),
            rd1_en=_has_src1(spec),
        )
        op.uops_sha[ver] = s.sha(ver)
    if name not in dve_ops._SUB_OPCODE_FOR_NAME:
        dve_ops.OPS.append(op)
        dve_ops._SUB_OPCODE_FOR_NAME[name] = (
            dve_ops._CUSTOM_DVE_ROW_BASE + len(dve_ops.OPS) - 1
        )
        dve_ops.CUSTOM_DVE_SPECS[name] = spec
    assert dve_ops._SUB_OPCODE_FOR_NAME[name] < 0x20
    _FR_OP = op
    return op


def _reg_const(nc, val):
    t = nc.alloc_sbuf_tensor(f"const-f32-{val}", [128, 1], F32)
    nc.gpsimd.memset(t.ap(), val)
    nc.const_aps.aps[(F32, float(val))] = t.ap()


def build():
    nc = bacc.Bacc()
    _reg_const(nc, SILU_BIAS_A)
    _reg_const(nc, SILU_BIAS_B)
    nc.all_engine_barrier()
    x_ext = nc.declare_dram_parameter("x", [NT, P, FD], F32, isOutput=False)
    o_ext = nc.declare_dram_parameter("out", [NT, P, FD], BF16, isOutput=True)

    with tile.TileContext(nc) as tc, tc.tile_pool(name="p", bufs=4) as pool:
        for i in range(NT):
            xt = pool.tile([P, FD], F32, tag="xt")
            nc.sync.dma_start(xt[:], x_ext[i])
            # u' = x*(8/7) + (64/7 - 0.5), bf16 out  (ACT Copy FMA)
            up = pool.tile([P, FD], BF16, tag="up")
            nc.scalar.activation(up[:], xt[:], Act.Copy, bias=UP_BIAS, scale=C87)
            # kfm = bf16(u' + 141) = 141 + floor(u)   (bf16 ulp-1 snap)
            kfm = pool.tile([P, FD], BF16, tag="kfm")
            nc.vector.tensor_single_scalar(kfm[:], up[:], MAGIC, Alu.add)
            # fr = (x*(8/7) + 64/7) - (kfm - 141) = u - k  (custom DVE, fp32 x)
            fr = pool.tile([P, FD], BF16, tag="fr")
            nc.vector._custom_dve(
                _get_fr_op(),
                out=fr[:],
                in0=xt[:],
                in1=kfm[:],
                s0=-MAGIC,
                s1=C87,
                imm2=C647,
            )
            # a = silu(t_k), b = silu(t_k + 0.875)   (bf16, free input FMA)
            a = pool.tile([P, FD], BF16, tag="a")
            nc.scalar.activation(a[:], kfm[:], Act.Silu, bias=SILU_BIAS_A, scale=0.875)
            b = pool.tile([P, FD], BF16, tag="b")
            nc.scalar.activation(b[:], kfm[:], Act.Silu, bias=SILU_BIAS_B, scale=0.875)
            # t = a - b   (bf16 2x)
            t = pool.tile([P, FD], BF16, tag="t")
            nc.vector.tensor_tensor(t[:], a[:], b[:], Alu.subtract)
            # g = t*fr = (a-b)*fr   (bf16 2x)
            g = pool.tile([P, FD], BF16, tag="g")
            nc.vector.tensor_tensor(g[:], t[:], fr[:], Alu.mult)
            # o = a - g = a + (b-a)*fr   (bf16 2x)
            o = pool.tile([P, FD], BF16, tag="o")
            nc.vector.tensor_tensor(o[:], a[:], g[:], Alu.subtract)
            nc.sync.dma_start(o_ext[i], o[:])
    nc.compile()
    return nc


_NC_CACHE = None


def _get_nc():
    global _NC_CACHE
    if _NC_CACHE is None:
        _NC_CACHE = build()
    return _NC_CACHE


def _ensure_ntff_hook():
    """Install the antenv.axon_hooks shim so trace=True works under axon."""
    import sys
    import types

    if "antenv.axon_hooks" not in sys.modules:
        mod = types.ModuleType("antenv.axon_hooks")
        _h = [None]
        mod.set_axon_ntff_profile_hook = lambda h: _h.__setitem__(0, h)
        mod.get_axon_ntff_profile_hook = lambda: _h[0]
        sys.modules["antenv.axon_hooks"] = mod
        import antenv

        antenv.axon_hooks = mod
    import antenv.axon_hooks as ah

    if ah.get_axon_ntff_profile_hook() is None:
        from trn_agent_boot.trn_boot import _ntff_profile_via_ctypes

        h = _ntff_profile_via_ctypes("/opt/axon/libaxon_pjrt.so")
        if h is not None:
            ah.set_axon_ntff_profile_hook(h)
    # avoid cloud artifact uploads in this container
    import concourse.bass_utils as bu

    bu.upload_artifacts = lambda tmpdir: tmpdir


def _run(x, trace=False, trace_kwargs=None):
    """x: (8, 2048, 4096) float32. Returns (out, exec_time_ns|None)."""
    x = np.ascontiguousarray(np.asarray(x, dtype=np.float32))
    assert x.shape == (N_CORES, 2048, 4096), x.shape
    nc = _get_nc()
    core_ids = list(range(N_CORES))
    in_maps = [{"x": x[i].reshape(NT, P, FD)} for i in range(N_CORES)]
    kwargs = {}
    if trace:
        _ensure_ntff_hook()
        kwargs["trace"] = True
        if trace_kwargs:
            kwargs.update(trace_kwargs)
    res = run_bass_kernel_spmd(nc, in_maps, core_ids, **kwargs)
    out = np.empty((N_CORES, 2048, 4096), dtype=np.float32)
    for i in range(N_CORES):
        out[i] = np.asarray(res.results[i]["out"], dtype=np.float32).reshape(
            2048, 4096
        )
    return out, res.exec_time_ns


def kernel(x, seg=None, silu_vals=None, **_unused):
    out, _ = _run(x, trace=False)
    return out


# revision 19
# speedup vs baseline: 2.1159x; 1.0523x over previous
"""ApproxSiLU16_FXP Trainium2 kernel (8 NeuronCores, data-parallel).

The reference computes a 16-segment piecewise-linear fixed-point
approximation of SiLU on a uniform knot grid t_k = -8 + 0.875k
(k = 0..16), with knot values y_k = round(1024*silu(t_k))/1024.
Instead of gathering from the LUT per element, this kernel
reconstructs the same piecewise-linear function analytically:

    u   = x*(8/7) + 64/7          (segment coordinate, in [0,16])
    k   = floor(u)
    fr  = u - k
    out = silu(t_k) + fr*(silu(t_k+0.875) - silu(t_k))

using the ScalarEngine's Silu activation for the knot values.  This
matches the fixed-point reference to ~2e-3 relative error (reference
LUT int rounding + fp16 rounding of the blend), well under the 2e-2
gate.

floor() uses a small-magic fp16 trick: u' = u - 0.5 rounded to fp16,
then kfm = fp16(u' + 1029).  1029 + [-0.5, 16.1] lies inside the fp16
binade [1024, 2048) where the fp16 ulp is exactly 1.0, so the output
rounding (DVE computes fp32 internally, rounds on the write) snaps to
1029 + floor(u).  Both Silus then use the free input FMA:
t_k = 0.875*kfm - 908.375 (exact in fp32).  fr = u' - (kfm - 1029.5)
needs one more fp16 scalar-add and one fp16 subtract, all in fast
16-bit DVE modes.

Engine split per tile (balanced ~183us each at FD=2048):
  ACT : u' (Copy FMA -> fp16, most tiles), a = silu(...), b = silu(...)
  DVE : kfm, kfm2 (fp16 1-scalar add, ~4x), fr, t, g, o (fp16 2x TT)
out = a - (a-b)*fr, stored fp16; the host upcasts to fp32.

Sharding: x is (8, 2048, 4096); core i processes batch row i.
"""

import numpy as np

from concourse import bacc, mybir
import concourse.tile as tile
from concourse.bass_utils import run_bass_kernel_spmd

F32 = mybir.dt.float32
F16 = mybir.dt.float16
Alu = mybir.AluOpType
Act = mybir.ActivationFunctionType

P = 128          # SBUF partitions
FD = 2048        # free dim per tile
NT = 32          # tiles per core shard: 2048*4096 = NT*P*FD
N_CORES = 8

C87 = float(8.0 / 7.0)
UP_BIAS = float(64.0 / 7.0 - 0.5)
MAGIC = 1029.0           # fp16 binade [1024,2048): ulp = 1
MAGIC2 = -1029.5         # kfm2 = k - 0.5
SILU_BIAS_A = -908.375   # 0.875*(k+1029) - 908.375 = 0.875k - 8 = t_k
SILU_BIAS_B = -907.5     # t_k + 0.875


def _reg_const(nc, val):
    t = nc.alloc_sbuf_tensor(f"const-f32-{val}", [128, 1], F32)
    nc.gpsimd.memset(t.ap(), val)
    nc.const_aps.aps[(F32, float(val))] = t.ap()


def build():
    nc = bacc.Bacc()
    _reg_const(nc, SILU_BIAS_A)
    _reg_const(nc, SILU_BIAS_B)
    nc.all_engine_barrier()
    x_ext = nc.declare_dram_parameter("x", [NT, P, FD], F32, isOutput=False)
    o_ext = nc.declare_dram_parameter("out", [NT, P, FD], F16, isOutput=True)

    with tile.TileContext(nc) as tc, tc.tile_pool(name="p", bufs=4) as pool:
        for i in range(NT):
            xt = pool.tile([P, FD], F32, tag="xt")
            nc.sync.dma_start(xt[:], x_ext[i])
            # u' = x*(8/7) + (64/7 - 0.5), fp16 out.  Mostly on ACT; a small
            # share on DVE to balance engine load.
            up = pool.tile([P, FD], F16, tag="up")
            if i % 8 == 7:
                nc.vector.tensor_scalar(
                    up[:], xt[:], C87, UP_BIAS, Alu.mult, Alu.add
                )
            else:
                nc.scalar.activation(up[:], xt[:], Act.Copy, bias=UP_BIAS, scale=C87)
            # kfm = fp16(u' + 1029) = 1029 + floor(u)   (fp16 ulp-1 snap)
            kfm = pool.tile([P, FD], F16, tag="kfm")
            nc.vector.tensor_single_scalar(kfm[:], up[:], MAGIC, Alu.add)
            # kfm2 = k - 0.5   (fp16 exact)
            kfm2 = pool.tile([P, FD], F16, tag="kfm2")
            nc.vector.tensor_single_scalar(kfm2[:], kfm[:], MAGIC2, Alu.add)
            # fr = u' - kfm2 = u - k   (fp16 2x)
            fr = pool.tile([P, FD], F16, tag="fr")
            nc.vector.tensor_tensor(fr[:], up[:], kfm2[:], Alu.subtract)
            # a = silu(t_k), b = silu(t_k + 0.875)   (fp16, free input FMA)
            a = pool.tile([P, FD], F16, tag="a")
            nc.scalar.activation(a[:], kfm[:], Act.Silu, bias=SILU_BIAS_A, scale=0.875)
            b = pool.tile([P, FD], F16, tag="b")
            nc.scalar.activation(b[:], kfm[:], Act.Silu, bias=SILU_BIAS_B, scale=0.875)
            # t = a - b   (fp16 2x)
            t = pool.tile([P, FD], F16, tag="t")
            nc.vector.tensor_tensor(t[:], a[:], b[:], Alu.subtract)
            # g = t*fr = (a-b)*fr   (fp16 2x)
            g = pool.tile([P, FD], F16, tag="g")
            nc.vector.tensor_tensor(g[:], t[:], fr[:], Alu.mult)
            # o = a - g = a + (b-a)*fr   (fp16 2x)
            o = pool.tile([P, FD], F16, tag="o")
            nc.vector.tensor_tensor(o[:], a[:], g[:], Alu.subtract)
            nc.sync.dma_start(o_ext[i], o[:])
    nc.compile()
    return nc


_NC_CACHE = None


def _get_nc():
    global _NC_CACHE
    if _NC_CACHE is None:
        _NC_CACHE = build()
    return _NC_CACHE


def _ensure_ntff_hook():
    """Install the antenv.axon_hooks shim so trace=True works under axon."""
    import sys
    import types

    if "antenv.axon_hooks" not in sys.modules:
        mod = types.ModuleType("antenv.axon_hooks")
        _h = [None]
        mod.set_axon_ntff_profile_hook = lambda h: _h.__setitem__(0, h)
        mod.get_axon_ntff_profile_hook = lambda: _h[0]
        sys.modules["antenv.axon_hooks"] = mod
        import antenv

        antenv.axon_hooks = mod
    import antenv.axon_hooks as ah

    if ah.get_axon_ntff_profile_hook() is None:
        from trn_agent_boot.trn_boot import _ntff_profile_via_ctypes

        h = _ntff_profile_via_ctypes("/opt/axon/libaxon_pjrt.so")
        if h is not None:
            ah.set_axon_ntff_profile_hook(h)
    # avoid cloud artifact uploads in this container
    import concourse.bass_utils as bu

    bu.upload_artifacts = lambda tmpdir: tmpdir


def _run_once(x, trace=False, trace_kwargs=None):
    nc = _get_nc()
    core_ids = list(range(N_CORES))
    in_maps = [{"x": x[i].reshape(NT, P, FD)} for i in range(N_CORES)]
    kwargs = {}
    if trace:
        _ensure_ntff_hook()
        kwargs["trace"] = True
        if trace_kwargs:
            kwargs.update(trace_kwargs)
    res = run_bass_kernel_spmd(nc, in_maps, core_ids, **kwargs)
    out = np.empty((N_CORES, 2048, 4096), dtype=np.float32)
    for i in range(N_CORES):
        out[i] = np.asarray(res.results[i]["out"], dtype=np.float32).reshape(
            2048, 4096
        )
    return out, res.exec_time_ns


def _run(x, trace=False, trace_kwargs=None):
    """x: (8, 2048, 4096) float32. Returns (out, exec_time_ns|None)."""
    x = np.ascontiguousarray(np.asarray(x, dtype=np.float32))
    assert x.shape == (N_CORES, 2048, 4096), x.shape
    # The axon terminal occasionally reports a transient unrecoverable
    # error on the first execution of a freshly loaded NEFF; retry.
    last_exc = None
    for _attempt in range(3):
        try:
            return _run_once(x, trace=trace, trace_kwargs=trace_kwargs)
        except Exception as e:  # noqa: BLE001
            last_exc = e
            import time

            time.sleep(2.0)
    raise last_exc


def kernel(x, seg=None, silu_vals=None, **_unused):
    out, _ = _run(x, trace=False)
    return out
